# revision 22
# baseline (speedup 1.0000x reference)
"""2-layer GAT on 8 Trainium2 NeuronCores (Bass/Tile).

Strategy (dst-partitioned, gather-based):
- Nodes are partitioned contiguously across 8 cores by destination; each core
  handles all edges whose dst lands in its range, so per-core outputs and the
  per-destination softmax segments are fully local (no cross-core reduction).
- Per layer, each core computes node rows [z | es | ed] for its own nodes with
  TensorE matmuls, the 8 slices are AllGather-ed into a replicated DRAM table,
  and each core uses `dma_gather` (512B rows) to fetch z/es of every edge's
  source node.
- Edges are laid out host-side in a (node-partition x slot) grid: each 128-node
  chunk gets T slot-tiles; tile t holds the t-th incoming edge of each node in
  partition p. Nodes are bucketed by in-degree (split into low/high source
  ranges for int16 gather indices, superblock-sorted on both counts) so
  padding is small. Pad slots point at a dummy table row with es = -1e30,
  which exp() maps to an exact 0 weight.
- Per slot-tile: ex = exp(leaky_relu(es_src + ed_dst)) on DVE/ACT written into
  a fused [pay | ex] bf16 tile, payload ex*z on DVE, then ONE
  identity-stationary matmul per tile accumulates the weighted sum and the
  softmax denominator together into PSUM. A per-chunk epilogue divides,
  applies elu (layer 1), and computes the next layer's node rows.
- Layer-1 table rows are 512B (z bf16 + es/ed f32); layer-2 rows are 256B
  (z2 bf16 + es2/ed2 f32), halving the second AllGather. Tables are Shared
  DRAM (fast collective path). Groups are processed largest-first so the
  drain into each AllGather is short. dma_gather is q7 desc-gen bound
  (~7.8 ns/row); prep/trigger splitting and >1024-idx calls do NOT help.
"""
import sys

sys.path.insert(0, "/opt/trn_rl_repo")

import numpy as np
import ml_dtypes

import concourse.bass as bass
import concourse.bacc as bacc
import concourse.mybir as mybir
import concourse.tile as tile
from concourse.library_config import mlp

F32 = mybir.dt.float32
BF16 = mybir.dt.bfloat16
I16 = mybir.dt.int16
AF = mybir.ActivationFunctionType
ALU = mybir.AluOpType
BF = ml_dtypes.bfloat16

C = 8          # cores
P = 128        # partitions


# --------------------------------------------------------------------------
# host-side preprocessing
# --------------------------------------------------------------------------

class Plan:
    """Host-computed layout shared by the program builder and per-core data."""


def build_plan(src, dst, n_nodes, group_chunks=None):
    import os
    if group_chunks is None:
        group_chunks = int(os.environ.get("GAT_GROUP", "2"))
    pl = Plan()
    npc = n_nodes // C
    assert npc * C == n_nodes
    chunks = -(-npc // P)
    npad = chunks * P
    slice_n = npad + 1              # + dummy row
    tbl_n = C * slice_n
    # low/high split for int16 gather indices
    hi_core = (C + 1) // 2          # cores [0,hi_core) low, rest high
    while hi_core * slice_n > 32768:
        hi_core -= 1
    assert (C - hi_core) * slice_n <= 32768, "table too large for 2-way split"
    hi_base = hi_core * slice_n
    pl.npc, pl.chunks, pl.npad = npc, chunks, npad
    pl.slice_n, pl.tbl_n, pl.hi_core, pl.hi_base = slice_n, tbl_n, hi_core, hi_base

    owner = dst // npc
    src_owner = src // npc
    is_lo = src_owner < hi_core

    # per-core, per-node in-edge lists split by src range
    perm = np.zeros((C, npad), np.int64)        # processing order -> local id
    klo = np.zeros((C, npad), np.int32)
    khi = np.zeros((C, npad), np.int32)
    edges_lo = []                                # per core: [n_lo_edges] srcs sorted by (dstlocal)
    edges_hi = []
    sb = int(os.environ.get("GAT_SB", "8")) * P  # superblock resort size
    for c in range(C):
        m = owner == c
        d_loc = dst[m] - c * npc
        s = src[m]
        lo_m = is_lo[m]
        cnt_lo = np.bincount(d_loc[lo_m], minlength=npc)
        cnt_hi = np.bincount(d_loc[~lo_m], minlength=npc)
        order = np.lexsort((cnt_hi, cnt_lo))     # sort nodes by (klo, khi)
        if sb > 0:
            # re-sort by khi within superblocks: keeps klo nearly sorted
            # (narrow range per block) while making khi sorted within each
            # block, shrinking both per-chunk maxima.
            kh_o = cnt_hi[order]
            for b in range(0, npc, sb):
                e = min(b + sb, npc)
                sub = np.argsort(kh_o[b:e], kind="stable")
                order[b:e] = order[b:e][sub]
        perm[c, :npc] = order
        perm[c, npc:] = npc                      # phantom marker
        klo[c, :npc] = cnt_lo[order]
        khi[c, :npc] = cnt_hi[order]
        # edge lists grouped by local dst: sort edges by d_loc
        o_lo = np.argsort(d_loc[lo_m], kind="stable")
        o_hi = np.argsort(d_loc[~lo_m], kind="stable")
        edges_lo.append((d_loc[lo_m][o_lo], s[lo_m][o_lo]))
        edges_hi.append((d_loc[~lo_m][o_hi], s[~lo_m][o_hi]))

    # global per-chunk tile counts
    kl = klo.reshape(C, chunks, P)
    kh = khi.reshape(C, chunks, P)
    tlo = np.maximum(kl.max(axis=(0, 2)), 1)     # [chunks], >= 1
    thi = kh.max(axis=(0, 2))                    # [chunks]
    pl.tlo, pl.thi = tlo, thi

    # groups of chunks per gather call, processed largest-first so the
    # epilogue drain before each AllGather is a small chunk's chain
    pl.group = group_chunks
    groups = [list(range(g, min(g + group_chunks, chunks)))
              for g in range(0, chunks, group_chunks)]
    tilesum = tlo + thi
    groups.sort(key=lambda g: -int(tilesum[g].sum()))
    pl.groups = groups

    # position of original node v in the table: owner*slice_n + invperm
    invperm = np.zeros((C, npc), np.int64)
    for c in range(C):
        invperm[c, perm[c, :npc]] = np.arange(npad)[: npc]
    pos = (owner_all := np.arange(n_nodes) // npc) * slice_n \
        + invperm[owner_all, np.arange(n_nodes) % npc]
    pl.pos = pos
    pl.perm = perm

    # build per-core int16 gather index arrays (tile-major inside groups)
    dummy_rel = npad                            # dummy row, relative to base
    idx_lo = np.full((C, int(tlo.sum()) * P), dummy_rel, np.int32)
    idx_hi = np.full((C, int(thi.sum()) * P), dummy_rel, np.int32)
    lo_tile_base = np.concatenate([[0], np.cumsum(tlo)])   # per chunk
    hi_tile_base = np.concatenate([[0], np.cumsum(thi)])
    for c in range(C):
        for (d_loc, s), karr, idx, tbase, tcnt, base_off in (
            (edges_lo[c], kl[c], idx_lo[c], lo_tile_base, tlo, 0),
            (edges_hi[c], kh[c], idx_hi[c], hi_tile_base, thi, pl.hi_base),
        ):
            if len(d_loc) == 0:
                continue
            posv = pos[s] - base_off
            # slot index of each edge within its node's list (0..k-1)
            # edges are sorted by d_loc; slot = running index within node
            slot = np.arange(len(d_loc)) - np.concatenate(
                [[0], np.cumsum(np.bincount(d_loc, minlength=npc))])[d_loc]
            # node -> (chunk, partition) via invperm
            ip = invperm[c, d_loc]
            ch, p = ip // P, ip % P
            flat = (tbase[ch] + slot) * P + p
            idx[flat] = posv
    assert idx_lo.max() < 32768 and idx_hi.max() < 32768
    pl.idx_lo, pl.idx_hi = idx_lo.astype(np.int16), idx_hi.astype(np.int16)
    pl.lo_tile_base, pl.hi_tile_base = lo_tile_base, hi_tile_base
    return pl


def wrap_idx(arr):
    """[n] int16 -> [128, n/16] wrapped + replicated across the 8 q7 cores."""
    n = arr.shape[0]
    assert n % 16 == 0
    w = arr.reshape(n // 16, 16).T               # [16, n/16]
    return np.tile(w, (8, 1)).copy()


# --------------------------------------------------------------------------
# device program
# --------------------------------------------------------------------------

def build_program(pl, in_dim, hid, heads, out_dim, num_devices=C):
    import os
    phase = os.environ.get("GAT_PHASE", "full")
    nd = heads * hid                 # 128 (layer-1 z width)
    assert nd == 128 and in_dim % P == 0
    kq = in_dim // P                 # k-chunks for layer-1 matmul
    chunks, npad, slice_n, tbl_n = pl.chunks, pl.npad, pl.slice_n, pl.tbl_n
    tlo, thi = pl.tlo, pl.thi
    ncols_lo = int(tlo.sum()) * 8    # idx sbuf cols
    ncols_hi = int(thi.sum()) * 8

    nocc = os.environ.get("GAT_NOCC", "0") == "1"
    maxidx = int(os.environ.get("GAT_MAXIDX", "1024"))
    scratch = int(os.environ.get("GAT_SCRATCH", "16384"))
    nc = bacc.Bacc("TRN2", target_bir_lowering=False, debug=False,
                   enable_asserts=False, num_devices=num_devices,
                   dynamic_dma_scratch_size=scratch)
    h_in = nc.dram_tensor("ht", [in_dim, npad], BF16, kind="ExternalInput")
    ilo_in = nc.dram_tensor("idx_lo", [P, max(ncols_lo, 8)], I16,
                            kind="ExternalInput")
    ihi_in = nc.dram_tensor("idx_hi", [P, max(ncols_hi, 8)], I16,
                            kind="ExternalInput")
    st1_in = nc.dram_tensor("stat1", [in_dim, 136], BF16,
                            kind="ExternalInput")
    st2_in = nc.dram_tensor("stat2", [nd, out_dim + 2], F32,
                            kind="ExternalInput")
    id_in = nc.dram_tensor("ident", [P, P], BF16, kind="ExternalInput")
    idf_in = nc.dram_tensor("identf", [P, P], F32, kind="ExternalInput")
    dum_in = nc.dram_tensor("dummyrow", [1, P], F32, kind="ExternalInput")
    out_dram = nc.dram_tensor("out", [npad, out_dim], F32,
                              kind="ExternalOutput")

    with tile.TileContext(nc) as tc:
        with (tc.tile_pool(name="const", bufs=1) as cpool,
              tc.tile_pool(name="dram", bufs=1, space="DRAM") as dpool,
              tc.tile_pool(name="work", bufs=3) as wpool,
              tc.tile_pool(name="gath", bufs=4) as gpool,
              tc.tile_pool(name="psA", bufs=1, space="PSUM") as pspool,
              tc.tile_pool(name="psE", bufs=3, space="PSUM") as pspoolE,
              tc.tile_pool(name="psZ", bufs=2, space="PSUM") as pspoolZ,
              tc.tile_pool(name="psB", bufs=1, space="PSUM") as pspool2):
            nc.gpsimd.load_library(mlp)

            # ---- constants / persistent tiles
            ident = cpool.tile([P, P], BF16)
            nc.sync.dma_start(ident[:], id_in[:])
            identf = cpool.tile([P, P], F32)
            nc.sync.dma_start(identf[:], idf_in[:])
            stat1 = cpool.tile([P, kq, 136], BF16)
            nc.sync.dma_start(
                stat1[:], st1_in.ap().rearrange("(q p) n -> p q n", p=P))
            stat2 = cpool.tile([P, out_dim + 2], F32)
            nc.sync.dma_start(stat2[:], st2_in[:])
            idx_lo = cpool.tile([P, max(ncols_lo, 8)], I16)
            nc.scalar.dma_start(idx_lo[:], ilo_in[:])
            idx_hi = cpool.tile([P, max(ncols_hi, 8)], I16)
            nc.scalar.dma_start(idx_hi[:], ihi_in[:])
            scores1 = cpool.tile([P, chunks, 8], F32)
            scores2 = cpool.tile([P, chunks], F32)

            slice1 = dpool.tile([slice_n, P], F32)
            table1 = dpool.tile([tbl_n, P], F32, addr_space="Shared")
            # layer-2 rows are 256B (z2 bf16 + es2/ed2 f32): half the
            # AllGather traffic of layer 1
            slice2 = dpool.tile([slice_n, 64], F32)
            table2 = dpool.tile([tbl_n, 64], F32, addr_space="Shared")

            # ---- phase Z1: own rows [z|es|ed] from host-transposed bf16 h
            # batch 4 chunks per DMA so the sync engine's issue+wait chain
            # (~2us per DMA) stops pacing the phase
            hT_view = h_in.ap().rearrange("(q p) n -> p q n", p=P)
            ZB = 3
            for k0 in range(0, chunks, ZB):
                zb = min(ZB, chunks - k0)
                hT = wpool.tile([P, kq, zb * P], BF16, tag="hT")
                nc.sync.dma_start(
                    hT[:], hT_view[:, :, k0 * P:(k0 + zb) * P])
                rowt = wpool.tile([P, zb, P], F32, tag="rowt")
                psz = pspoolZ.tile([P, zb, 136], F32, tag="psz")
                for j in range(zb):
                    for q in range(kq):
                        nc.tensor.matmul(psz[:, j, :],
                                         hT[:, q, j * P:(j + 1) * P],
                                         stat1[:, q, :],
                                         start=(q == 0), stop=(q == kq - 1))
                nc.vector.memset(rowt[:, :, 72:P], 0.0)
                rbf = rowt.bitcast(BF16)
                nc.vector.tensor_copy(rbf[:, :, 0:P],
                                      psz[:, :, 0:P])
                nc.scalar.copy(rowt[:, :, 64:72], psz[:, :, 128:136])
                nc.vector.tensor_copy(scores1[:, k0:k0 + zb, :],
                                      psz[:, :, 128:136])
                nc.sync.dma_start(
                    slice1[k0 * P:(k0 + zb) * P, :]
                    .rearrange("(c p) n -> p c n", p=P),
                    rowt[:])
            nc.sync.dma_start(slice1[npad:npad + 1, :], dum_in[:])
            if nocc:
                pass
            else:
                nc.gpsimd.collective_compute(
                    "AllGather", ALU.bypass,
                    replica_groups=[list(range(num_devices))],
                    ins=[slice1[0:slice_n, :].opt()],
                    outs=[table1[0:tbl_n, :].opt()])

            elvl = int(os.environ.get("GAT_ELVL", "9"))

            # ---- edge phases
            def edge_phase(layer, table, scores_t):
                pay_w = P if layer == 1 else out_dim     # payload cols
                nh = 4 if layer == 1 else 1              # heads
                mw = pay_w + nh                          # payload + ex cols
                elem = 256 if layer == 1 else 128        # gathered bf16/row
                tbl_bf = table.bitcast(BF16)
                lo_src = tbl_bf[0:pl.hi_base, :]
                hi_src = tbl_bf[pl.hi_base:tbl_n, :]
                for grp in pl.groups:
                    nlo = int(tlo[grp].sum())
                    nhi = int(thi[grp].sum())
                    gt = gpool.tile([P, nlo + nhi, elem], BF16, tag="gt")
                    mt = maxidx // P
                    for (src_ap, idxt, base_t, n_t, dst0) in (
                            (lo_src, idx_lo, int(pl.lo_tile_base[grp[0]]),
                             nlo, 0),
                            (hi_src, idx_hi, int(pl.hi_tile_base[grp[0]]),
                             nhi, nlo)):
                        done = 0
                        while done < n_t:
                            nt = min(n_t - done, mt)
                            c0 = (base_t + done) * 8
                            nc.gpsimd.dma_gather(
                                gt[:, dst0 + done:dst0 + done + nt, :],
                                src_ap, idxt[:, c0:c0 + nt * 8],
                                nt * P, nt * P, elem)
                            done += nt
                    gt32 = gt.bitcast(F32)
                    lo_b = int(pl.lo_tile_base[grp[0]])
                    hi_b = int(pl.hi_tile_base[grp[0]])
                    if elvl == 0:
                        sink = wpool.tile([P, 64], F32, tag="sink")
                        nc.vector.tensor_copy(sink[:], gt32[:, 0, 0:64])
                        nc.sync.dma_start(
                            slice2[grp[0] * P:(grp[0] + 1) * P, :], sink[:])
                        continue
                    for k in grp:
                        tl, th = int(tlo[k]), int(thi[k])
                        T = tl + th
                        ko_lo = int(pl.lo_tile_base[k]) - lo_b
                        ko_hi = nlo + int(pl.hi_tile_base[k]) - hi_b
                        # e = es[src] + ed[dst]
                        e32 = wpool.tile([P, T, nh], F32, tag="e32")
                        for (off, cnt, eo) in ((ko_lo, tl, 0), (ko_hi, th, tl)):
                            if cnt == 0:
                                continue
                            if layer == 1:
                                esv = gt32[:, off:off + cnt, 64:68]
                                edv = (scores_t[:, k, 4:8].unsqueeze(1)
                                       .broadcast_to([P, cnt, 4]))
                            else:
                                esv = gt32[:, off:off + cnt, 32:33]
                                edv = (scores_t[:, k:k + 1].unsqueeze(1)
                                       .broadcast_to([P, cnt, 1]))
                            nc.vector.tensor_tensor(
                                e32[:, eo:eo + cnt, :], esv, edv, ALU.add)
                        ef = e32[:].rearrange("p t h -> p (t h)")
                        lr = wpool.tile([P, T, nh], F32, tag="lr")
                        lrf = lr[:].rearrange("p t h -> p (t h)")
                        nc.vector.scalar_tensor_tensor(
                            lrf, ef, 0.01, ef, ALU.mult, ALU.max)
                        # payex: [pay | ex] so one matmul accumulates the
                        # weighted sum and the softmax denominator together
                        payex = wpool.tile([P, T, mw], BF16, tag="payex")
                        nc.scalar.activation(
                            payex[:, 0:T, pay_w:mw], lr[:], AF.Exp)
                        if elvl == 1:
                            sink = wpool.tile([P, 64], F32, tag="sink")
                            nc.vector.memset(sink[:], 0.0)
                            nc.sync.dma_start(
                                slice2[k * P:(k + 1) * P, :], sink[:])
                            continue
                        psz = pspoolE.tile([P, mw], F32, tag="psE")
                        # two passes: all DVE multiplies first, then all
                        # matmuls — keeps DVE of chunk k+1 overlapping the
                        # TensorE accumulation of chunk k
                        for t in range(T):
                            col = (ko_lo + t) if t < tl else (ko_hi + t - tl)
                            if layer == 1:
                                zin = gt[:, col, 0:P].rearrange(
                                    "p (a b) -> p a b", a=4)
                                exv = (payex[:, t, pay_w:mw].unsqueeze(2)
                                       .broadcast_to([P, 4, 32]))
                                nc.vector.tensor_tensor(
                                    payex[:, t, 0:pay_w].rearrange(
                                        "p (a b) -> p a b", a=4),
                                    zin, exv, ALU.mult)
                            else:
                                zin = gt[:, col, 0:out_dim]
                                exv = (payex[:, t, pay_w:mw]
                                       .broadcast_to([P, out_dim]))
                                nc.vector.tensor_tensor(
                                    payex[:, t, 0:pay_w], zin, exv, ALU.mult)
                        for t in range(T):
                            nc.tensor.matmul(psz[:], ident[:], payex[:, t, :],
                                             start=(t == 0), stop=(t == T - 1))
                        # epilogue
                        den = wpool.tile([P, nh], F32, tag="den")
                        nc.vector.tensor_scalar_add(den[:], psz[:, pay_w:mw],
                                                    1e-30)
                        rec = wpool.tile([P, nh], F32, tag="rec")
                        nc.vector.reciprocal(rec[:], den[:])
                        if layer == 1:
                            h1 = wpool.tile([P, P], F32, tag="h1")
                            rv = (rec[:].unsqueeze(2)
                                  .broadcast_to([P, 4, 32]))
                            nc.vector.tensor_tensor(
                                h1[:].rearrange("p (a b) -> p a b", a=4),
                                psz[:, 0:P].rearrange("p (a b) -> p a b", a=4),
                                rv, ALU.mult)
                            # elu(x) = max(x, exp(min(x,0)) - 1)
                            mn = wpool.tile([P, P], F32, tag="mn")
                            nc.vector.tensor_scalar_min(mn[:], h1[:], 0.0)
                            em = wpool.tile([P, P], F32, tag="em")
                            nc.scalar.activation(em[:], mn[:], AF.Exp)
                            h1e = wpool.tile([P, P], F32, tag="h1e")
                            nc.vector.scalar_tensor_tensor(
                                h1e[:], em[:], -1.0, h1[:], ALU.add, ALU.max)
                            # z2 = h1e @ [W2 | W2 a2s | W2 a2d]
                            pst2 = pspool.tile([P, P], F32, tag="pst2")
                            nc.tensor.matmul(pst2[:], h1e[:], identf[:],
                                             is_transpose=True)
                            h1T = wpool.tile([P, P], F32, tag="h1T")
                            nc.vector.tensor_copy(h1T[:], pst2[:])
                            psz2 = pspool2.tile([P, out_dim + 2], F32,
                                                tag="psz2")
                            nc.tensor.matmul(psz2[:], h1T[:], stat2[:])
                            # layer-2 row: z2 as bf16 in bytes [0,128),
                            # es2/ed2 f32 at f32 cols 32:34 (256B rows)
                            row2 = wpool.tile([P, 64], F32, tag="row2")
                            r2bf = row2.bitcast(BF16)
                            nc.vector.tensor_copy(r2bf[:, 0:out_dim],
                                                  psz2[:, 0:out_dim])
                            nc.scalar.copy(row2[:, 32:34],
                                           psz2[:, out_dim:out_dim + 2])
                            nc.vector.tensor_copy(scores2[:, k:k + 1],
                                                  psz2[:, out_dim + 1:
                                                       out_dim + 2])
                            nc.sync.dma_start(slice2[k * P:(k + 1) * P, 0:34],
                                              row2[:, 0:34])
                        else:
                            orow = wpool.tile([P, out_dim], F32, tag="orow")
                            rv = rec[:].broadcast_to([P, out_dim])
                            nc.vector.tensor_tensor(orow[:], psz[:, 0:out_dim],
                                                    rv, ALU.mult)
                            nc.sync.dma_start(
                                out_dram[k * P:(k + 1) * P, :], orow[:])

            if phase == "z1":
                nc.sync.dma_start(out_dram[0:npad, :],
                                  slice1[0:npad, 0:out_dim])
            elif phase == "e1":
                edge_phase(1, table1, scores1)
                nc.sync.dma_start(out_dram[0:npad, :],
                                  slice2[0:npad, 0:out_dim])
            else:
                edge_phase(1, table1, scores1)
                nc.sync.dma_start(slice2[npad:npad + 1, :], dum_in[:, 0:64])
                if nocc:
                    pass
                else:
                    nc.gpsimd.collective_compute(
                        "AllGather", ALU.bypass,
                        replica_groups=[list(range(num_devices))],
                        ins=[slice2[0:slice_n, :].opt()],
                        outs=[table2[0:tbl_n, :].opt()])
                edge_phase(2, table2, scores2)

    nc.compile()
    return nc


# --------------------------------------------------------------------------
# host wrapper
# --------------------------------------------------------------------------

def make_inputs(pl, h, W1, a1, W2, a2, in_dim, hid, heads, out_dim):
    n_nodes = h.shape[0]
    npc, npad = pl.npc, pl.npad
    nd = heads * hid
    # stationaries
    st1 = np.zeros((in_dim, 136), np.float32)
    for hh in range(heads):
        st1[:, hh * hid:(hh + 1) * hid] = W1[hh]
        st1[:, 128 + hh] = W1[hh] @ a1[hh, :hid]
        st1[:, 132 + hh] = W1[hh] @ a1[hh, hid:]
    st2 = np.zeros((nd, out_dim + 2), np.float32)
    st2[:, :out_dim] = W2[0]
    st2[:, out_dim] = W2[0] @ a2[0, :out_dim]
    st2[:, out_dim + 1] = W2[0] @ a2[0, out_dim:]
    dummy = np.zeros((1, 128), np.float32)
    dummy[0, 64:72] = -1e30
    dummy[0, 32] = -1e30   # layer-2 es slot (256B rows)
    identb = np.eye(128, dtype=BF)
    identf = np.eye(128, dtype=np.float32)

    ncols_lo = int(pl.tlo.sum()) * 8
    ncols_hi = int(pl.thi.sum()) * 8
    in_maps = []
    st1b = st1.astype(BF)
    for c in range(C):
        hp = np.zeros((npad, in_dim), np.float32)
        valid = pl.perm[c] < npc
        hp[valid] = h[c * npc + pl.perm[c][valid]]
        ht = np.ascontiguousarray(hp.T).astype(BF)
        ilo = wrap_idx(pl.idx_lo[c]) if ncols_lo else \
            np.zeros((128, 8), np.int16)
        ihi = wrap_idx(pl.idx_hi[c]) if ncols_hi else \
            np.zeros((128, 8), np.int16)
        in_maps.append({
            "ht": ht, "idx_lo": ilo, "idx_hi": ihi,
            "stat1": st1b, "stat2": st2, "ident": identb, "identf": identf,
            "dummyrow": dummy,
        })
    return in_maps


def unpermute(pl, outs, n_nodes, out_dim):
    npc = pl.npc
    full = np.zeros((n_nodes, out_dim), np.float32)
    for c in range(C):
        valid = pl.perm[c] < npc
        full[c * npc + pl.perm[c][valid]] = outs[c][valid]
    return full


def gat_run(h, src, dst, W1, a1, W2, a2, runner):
    """Full pipeline; `runner(nc, in_maps)` -> list of per-core {'out': arr}."""
    n_nodes, in_dim = h.shape
    heads, _, hid = W1.shape
    out_dim = W2.shape[2]
    pl = build_plan(np.asarray(src), np.asarray(dst), n_nodes)
    nc = build_program(pl, in_dim, hid, heads, out_dim)
    in_maps = make_inputs(pl, np.asarray(h, np.float32), np.asarray(W1),
                          np.asarray(a1), np.asarray(W2), np.asarray(a2),
                          in_dim, hid, heads, out_dim)
    outs = runner(nc, in_maps)
    return unpermute(pl, [o["out"] for o in outs], n_nodes, out_dim)


def hw_runner(nc, in_maps, trace=None):
    import os
    from concourse.bass_utils import run_bass_kernel_spmd
    if trace is None:
        trace = os.environ.get("GAT_TRACE", "0") == "1"
    res = run_bass_kernel_spmd(nc, in_maps, core_ids=list(range(C)),
                               trace=trace)
    hw_runner.last = res
    return res.results


def kernel(**inputs):
    out = gat_run(inputs["h"], inputs["src"], inputs["dst"], inputs["W1"],
                  inputs["a1"], inputs["W2"], inputs["a2"], hw_runner)
    return out



# revision 23
# speedup vs baseline: 1.1806x; 1.1806x over previous
"""2-layer GAT on 8 Trainium2 NeuronCores (Bass/Tile).

Strategy (dst-partitioned, gather-based):
- Nodes are partitioned contiguously across 8 cores by destination; each core
  handles all edges whose dst lands in its range, so per-core outputs and the
  per-destination softmax segments are fully local (no cross-core reduction).
- Per layer, each core computes node rows [z | es | ed] for its own nodes with
  TensorE matmuls, the 8 slices are AllGather-ed into a replicated DRAM table,
  and each core uses `dma_gather` (512B rows) to fetch z/es of every edge's
  source node.
- Edges are laid out host-side in a (node-partition x slot) grid: each 128-node
  chunk gets T slot-tiles; tile t holds the t-th incoming edge of each node in
  partition p. Nodes are bucketed by in-degree (split into low/high source
  ranges for int16 gather indices, superblock-sorted on both counts) so
  padding is small. Pad slots point at a dummy table row with es = -1e30,
  which exp() maps to an exact 0 weight.
- Per slot-tile: ex = exp(leaky_relu(es_src + ed_dst)) on DVE/ACT written into
  a fused [pay | ex] bf16 tile, payload ex*z on DVE, then ONE
  identity-stationary matmul per tile accumulates the weighted sum and the
  softmax denominator together into PSUM. A per-chunk epilogue divides,
  applies elu (layer 1), and computes the next layer's node rows.
- Layer-1 table rows are 512B (z bf16 + es/ed f32); layer-2 rows are 256B
  (z2 bf16 + es2/ed2 f32), halving the second AllGather. Tables are Shared
  DRAM (fast collective path). Groups are processed largest-first so the
  drain into each AllGather is short. dma_gather is q7 desc-gen bound
  (~7.8 ns/row); prep/trigger splitting and >1024-idx calls do NOT help.
"""
import sys

sys.path.insert(0, "/opt/trn_rl_repo")

import numpy as np
import ml_dtypes

import concourse.bass as bass
import concourse.bacc as bacc
import concourse.mybir as mybir
import concourse.tile as tile
from concourse.library_config import mlp

F32 = mybir.dt.float32
BF16 = mybir.dt.bfloat16
I16 = mybir.dt.int16
AF = mybir.ActivationFunctionType
ALU = mybir.AluOpType
BF = ml_dtypes.bfloat16

C = 8          # cores
P = 128        # partitions


# --------------------------------------------------------------------------
# host-side preprocessing
# --------------------------------------------------------------------------

class Plan:
    """Host-computed layout shared by the program builder and per-core data."""


def build_plan(src, dst, n_nodes, group_chunks=None):
    import os
    if group_chunks is None:
        group_chunks = int(os.environ.get("GAT_GROUP", "2"))
    pl = Plan()
    npc = n_nodes // C
    assert npc * C == n_nodes
    chunks = -(-npc // P)
    npad = chunks * P
    slice_n = npad + 1              # + dummy row
    tbl_n = C * slice_n
    # low/high split for int16 gather indices
    hi_core = (C + 1) // 2          # cores [0,hi_core) low, rest high
    while hi_core * slice_n > 32768:
        hi_core -= 1
    assert (C - hi_core) * slice_n <= 32768, "table too large for 2-way split"
    hi_base = hi_core * slice_n
    pl.npc, pl.chunks, pl.npad = npc, chunks, npad
    pl.slice_n, pl.tbl_n, pl.hi_core, pl.hi_base = slice_n, tbl_n, hi_core, hi_base

    owner = dst // npc
    src_owner = src // npc
    is_lo = src_owner < hi_core

    # per-core, per-node in-edge lists split by src range
    perm = np.zeros((C, npad), np.int64)        # processing order -> local id
    klo = np.zeros((C, npad), np.int32)
    khi = np.zeros((C, npad), np.int32)
    edges_lo = []                                # per core: [n_lo_edges] srcs sorted by (dstlocal)
    edges_hi = []
    sb = int(os.environ.get("GAT_SB", "8")) * P  # superblock resort size
    for c in range(C):
        m = owner == c
        d_loc = dst[m] - c * npc
        s = src[m]
        lo_m = is_lo[m]
        cnt_lo = np.bincount(d_loc[lo_m], minlength=npc)
        cnt_hi = np.bincount(d_loc[~lo_m], minlength=npc)
        order = np.lexsort((cnt_hi, cnt_lo))     # sort nodes by (klo, khi)
        if sb > 0:
            # re-sort by khi within superblocks: keeps klo nearly sorted
            # (narrow range per block) while making khi sorted within each
            # block, shrinking both per-chunk maxima.
            kh_o = cnt_hi[order]
            for b in range(0, npc, sb):
                e = min(b + sb, npc)
                sub = np.argsort(kh_o[b:e], kind="stable")
                order[b:e] = order[b:e][sub]
        perm[c, :npc] = order
        perm[c, npc:] = npc                      # phantom marker
        klo[c, :npc] = cnt_lo[order]
        khi[c, :npc] = cnt_hi[order]
        # edge lists grouped by local dst: sort edges by d_loc
        o_lo = np.argsort(d_loc[lo_m], kind="stable")
        o_hi = np.argsort(d_loc[~lo_m], kind="stable")
        edges_lo.append((d_loc[lo_m][o_lo], s[lo_m][o_lo]))
        edges_hi.append((d_loc[~lo_m][o_hi], s[~lo_m][o_hi]))

    # global per-chunk tile counts
    kl = klo.reshape(C, chunks, P)
    kh = khi.reshape(C, chunks, P)
    tlo = np.maximum(kl.max(axis=(0, 2)), 1)     # [chunks], >= 1
    thi = kh.max(axis=(0, 2))                    # [chunks]
    pl.tlo, pl.thi = tlo, thi

    # groups of chunks per gather call, processed largest-first so the
    # epilogue drain before each AllGather is a small chunk's chain
    pl.group = group_chunks
    groups = [list(range(g, min(g + group_chunks, chunks)))
              for g in range(0, chunks, group_chunks)]
    tilesum = tlo + thi
    groups.sort(key=lambda g: -int(tilesum[g].sum()))
    pl.groups = groups

    # position of original node v in the table: owner*slice_n + invperm
    invperm = np.zeros((C, npc), np.int64)
    for c in range(C):
        invperm[c, perm[c, :npc]] = np.arange(npad)[: npc]
    pos = (owner_all := np.arange(n_nodes) // npc) * slice_n \
        + invperm[owner_all, np.arange(n_nodes) % npc]
    pl.pos = pos
    pl.perm = perm

    # build per-core int16 gather index arrays (tile-major inside groups)
    dummy_rel = npad                            # dummy row, relative to base
    idx_lo = np.full((C, int(tlo.sum()) * P), dummy_rel, np.int32)
    idx_hi = np.full((C, int(thi.sum()) * P), dummy_rel, np.int32)
    lo_tile_base = np.concatenate([[0], np.cumsum(tlo)])   # per chunk
    hi_tile_base = np.concatenate([[0], np.cumsum(thi)])
    for c in range(C):
        for (d_loc, s), karr, idx, tbase, tcnt, base_off in (
            (edges_lo[c], kl[c], idx_lo[c], lo_tile_base, tlo, 0),
            (edges_hi[c], kh[c], idx_hi[c], hi_tile_base, thi, pl.hi_base),
        ):
            if len(d_loc) == 0:
                continue
            posv = pos[s] - base_off
            # slot index of each edge within its node's list (0..k-1)
            # edges are sorted by d_loc; slot = running index within node
            slot = np.arange(len(d_loc)) - np.concatenate(
                [[0], np.cumsum(np.bincount(d_loc, minlength=npc))])[d_loc]
            # node -> (chunk, partition) via invperm
            ip = invperm[c, d_loc]
            ch, p = ip // P, ip % P
            flat = (tbase[ch] + slot) * P + p
            idx[flat] = posv
    assert idx_lo.max() < 32768 and idx_hi.max() < 32768
    pl.idx_lo, pl.idx_hi = idx_lo.astype(np.int16), idx_hi.astype(np.int16)
    pl.lo_tile_base, pl.hi_tile_base = lo_tile_base, hi_tile_base
    return pl


def wrap_idx(arr):
    """[n] int16 -> [128, n/16] wrapped + replicated across the 8 q7 cores."""
    n = arr.shape[0]
    assert n % 16 == 0
    w = arr.reshape(n // 16, 16).T               # [16, n/16]
    return np.tile(w, (8, 1)).copy()


# --------------------------------------------------------------------------
# device program
# --------------------------------------------------------------------------

def build_program(pl, in_dim, hid, heads, out_dim, num_devices=C):
    import os
    phase = os.environ.get("GAT_PHASE", "full")
    nd = heads * hid                 # 128 (layer-1 z width)
    assert nd == 128 and in_dim % P == 0
    kq = in_dim // P                 # k-chunks for layer-1 matmul
    chunks, npad, slice_n, tbl_n = pl.chunks, pl.npad, pl.slice_n, pl.tbl_n
    tlo, thi = pl.tlo, pl.thi
    ncols_lo = int(tlo.sum()) * 8    # idx sbuf cols
    ncols_hi = int(thi.sum()) * 8

    nocc = os.environ.get("GAT_NOCC", "0") == "1"
    maxidx = int(os.environ.get("GAT_MAXIDX", "1024"))
    scratch = int(os.environ.get("GAT_SCRATCH", "16384"))
    nc = bacc.Bacc("TRN2", target_bir_lowering=False, debug=False,
                   enable_asserts=False, num_devices=num_devices,
                   dynamic_dma_scratch_size=scratch)
    h_in = nc.dram_tensor("ht", [in_dim, npad], BF16, kind="ExternalInput")
    ilo_in = nc.dram_tensor("idx_lo", [P, max(ncols_lo, 8)], I16,
                            kind="ExternalInput")
    ihi_in = nc.dram_tensor("idx_hi", [P, max(ncols_hi, 8)], I16,
                            kind="ExternalInput")
    st1_in = nc.dram_tensor("stat1", [in_dim, 136], BF16,
                            kind="ExternalInput")
    st2_in = nc.dram_tensor("stat2", [nd, out_dim + 2], F32,
                            kind="ExternalInput")
    id_in = nc.dram_tensor("ident", [P, P], BF16, kind="ExternalInput")
    idf_in = nc.dram_tensor("identf", [P, P], F32, kind="ExternalInput")
    dum_in = nc.dram_tensor("dummyrow", [1, P], F32, kind="ExternalInput")
    out_dram = nc.dram_tensor("out", [npad, out_dim], F32,
                              kind="ExternalOutput")

    with tile.TileContext(nc) as tc:
        with (tc.tile_pool(name="const", bufs=1) as cpool,
              tc.tile_pool(name="dram", bufs=1, space="DRAM") as dpool,
              tc.tile_pool(name="work", bufs=3) as wpool,
              tc.tile_pool(name="gath", bufs=3) as gpool,
              tc.tile_pool(name="psA", bufs=1, space="PSUM") as pspool,
              tc.tile_pool(name="psE", bufs=2, space="PSUM") as pspoolE,
              tc.tile_pool(name="psZ", bufs=2, space="PSUM") as pspoolZ,
              tc.tile_pool(name="psB", bufs=1, space="PSUM") as pspool2):
            nc.gpsimd.load_library(mlp)

            # ---- constants / persistent tiles
            ident = cpool.tile([P, P], BF16)
            nc.sync.dma_start(ident[:], id_in[:])
            identf = cpool.tile([P, P], F32)
            nc.sync.dma_start(identf[:], idf_in[:])
            stat1 = cpool.tile([P, kq, 136], BF16)
            nc.sync.dma_start(
                stat1[:], st1_in.ap().rearrange("(q p) n -> p q n", p=P))
            stat2 = cpool.tile([P, out_dim + 2], F32)
            nc.sync.dma_start(stat2[:], st2_in[:])
            idx_lo = cpool.tile([P, max(ncols_lo, 8)], I16)
            nc.scalar.dma_start(idx_lo[:], ilo_in[:])
            idx_hi = cpool.tile([P, max(ncols_hi, 8)], I16)
            nc.scalar.dma_start(idx_hi[:], ihi_in[:])
            scores1 = cpool.tile([P, chunks, 8], F32)
            scores2 = cpool.tile([P, chunks], F32)

            slice1 = dpool.tile([slice_n, P], F32)
            table1 = dpool.tile([tbl_n, P], F32, addr_space="Shared")
            # layer-2 rows are 256B (z2 bf16 + es2/ed2 f32): half the
            # AllGather traffic of layer 1
            slice2 = dpool.tile([slice_n, 64], F32)
            table2 = dpool.tile([tbl_n, 64], F32, addr_space="Shared")

            # ---- phase Z1: own rows [z|es|ed] from host-transposed bf16 h
            # batch 4 chunks per DMA so the sync engine's issue+wait chain
            # (~2us per DMA) stops pacing the phase
            hT_view = h_in.ap().rearrange("(q p) n -> p q n", p=P)
            ZB = 3
            for k0 in range(0, chunks, ZB):
                zb = min(ZB, chunks - k0)
                hT = wpool.tile([P, kq, zb * P], BF16, tag="hT")
                nc.sync.dma_start(
                    hT[:], hT_view[:, :, k0 * P:(k0 + zb) * P])
                rowt = wpool.tile([P, zb, P], F32, tag="rowt")
                psz = pspoolZ.tile([P, zb, 136], F32, tag="psz")
                for j in range(zb):
                    for q in range(kq):
                        nc.tensor.matmul(psz[:, j, :],
                                         hT[:, q, j * P:(j + 1) * P],
                                         stat1[:, q, :],
                                         start=(q == 0), stop=(q == kq - 1))
                nc.vector.memset(rowt[:, :, 72:P], 0.0)
                rbf = rowt.bitcast(BF16)
                nc.vector.tensor_copy(rbf[:, :, 0:P],
                                      psz[:, :, 0:P])
                nc.scalar.copy(rowt[:, :, 64:72], psz[:, :, 128:136])
                nc.vector.tensor_copy(scores1[:, k0:k0 + zb, :],
                                      psz[:, :, 128:136])
                nc.sync.dma_start(
                    slice1[k0 * P:(k0 + zb) * P, :]
                    .rearrange("(c p) n -> p c n", p=P),
                    rowt[:])
            nc.sync.dma_start(slice1[npad:npad + 1, :], dum_in[:])
            if nocc:
                pass
            else:
                nc.gpsimd.collective_compute(
                    "AllGather", ALU.bypass,
                    replica_groups=[list(range(num_devices))],
                    ins=[slice1[0:slice_n, :].opt()],
                    outs=[table1[0:tbl_n, :].opt()])

            elvl = int(os.environ.get("GAT_ELVL", "9"))

            # ---- edge phases
            def edge_phase(layer, table, scores_t):
                pay_w = P if layer == 1 else out_dim     # payload cols
                nh = 4 if layer == 1 else 1              # heads
                mw = pay_w + nh                          # payload + ex cols
                elem = 256 if layer == 1 else 128        # gathered bf16/row
                tbl_bf = table.bitcast(BF16)
                lo_src = tbl_bf[0:pl.hi_base, :]
                hi_src = tbl_bf[pl.hi_base:tbl_n, :]
                for grp in pl.groups:
                    nlo = int(tlo[grp].sum())
                    nhi = int(thi[grp].sum())
                    gt = gpool.tile([P, nlo + nhi, elem], BF16, tag="gt")
                    mt = maxidx // P
                    for (src_ap, idxt, base_t, n_t, dst0) in (
                            (lo_src, idx_lo, int(pl.lo_tile_base[grp[0]]),
                             nlo, 0),
                            (hi_src, idx_hi, int(pl.hi_tile_base[grp[0]]),
                             nhi, nlo)):
                        done = 0
                        while done < n_t:
                            nt = min(n_t - done, mt)
                            c0 = (base_t + done) * 8
                            nc.gpsimd.dma_gather(
                                gt[:, dst0 + done:dst0 + done + nt, :],
                                src_ap, idxt[:, c0:c0 + nt * 8],
                                nt * P, nt * P, elem)
                            done += nt
                    gt32 = gt.bitcast(F32)
                    lo_b = int(pl.lo_tile_base[grp[0]])
                    hi_b = int(pl.hi_tile_base[grp[0]])
                    if elvl == 0:
                        sink = wpool.tile([P, 64], F32, tag="sink")
                        nc.vector.tensor_copy(sink[:], gt32[:, 0, 0:64])
                        nc.sync.dma_start(
                            slice2[grp[0] * P:(grp[0] + 1) * P, :], sink[:])
                        continue
                    for k in grp:
                        tl, th = int(tlo[k]), int(thi[k])
                        T = tl + th
                        ko_lo = int(pl.lo_tile_base[k]) - lo_b
                        ko_hi = nlo + int(pl.hi_tile_base[k]) - hi_b
                        # e = es[src] + ed[dst]
                        e32 = wpool.tile([P, T, nh], F32, tag="e32")
                        for (off, cnt, eo) in ((ko_lo, tl, 0), (ko_hi, th, tl)):
                            if cnt == 0:
                                continue
                            if layer == 1:
                                esv = gt32[:, off:off + cnt, 64:68]
                                edv = (scores_t[:, k, 4:8].unsqueeze(1)
                                       .broadcast_to([P, cnt, 4]))
                            else:
                                esv = gt32[:, off:off + cnt, 32:33]
                                edv = (scores_t[:, k:k + 1].unsqueeze(1)
                                       .broadcast_to([P, cnt, 1]))
                            nc.vector.tensor_tensor(
                                e32[:, eo:eo + cnt, :], esv, edv, ALU.add)
                        ef = e32[:].rearrange("p t h -> p (t h)")
                        lr = wpool.tile([P, T, nh], F32, tag="lr")
                        lrf = lr[:].rearrange("p t h -> p (t h)")
                        nc.vector.scalar_tensor_tensor(
                            lrf, ef, 0.01, ef, ALU.mult, ALU.max)
                        # payex: [pay | ex] so one matmul accumulates the
                        # weighted sum and the softmax denominator together
                        payex = wpool.tile([P, T, mw], BF16, tag="payex")
                        nc.scalar.activation(
                            payex[:, 0:T, pay_w:mw], lr[:], AF.Exp)
                        if elvl == 1:
                            sink = wpool.tile([P, 64], F32, tag="sink")
                            nc.vector.memset(sink[:], 0.0)
                            nc.sync.dma_start(
                                slice2[k * P:(k + 1) * P, :], sink[:])
                            continue
                        psz = pspoolE.tile([P, mw], F32, tag="psE")
                        # two passes: all DVE multiplies first, then all
                        # matmuls — keeps DVE of chunk k+1 overlapping the
                        # TensorE accumulation of chunk k
                        for t in range(T):
                            col = (ko_lo + t) if t < tl else (ko_hi + t - tl)
                            if layer == 1:
                                zin = gt[:, col, 0:P].rearrange(
                                    "p (a b) -> p a b", a=4)
                                exv = (payex[:, t, pay_w:mw].unsqueeze(2)
                                       .broadcast_to([P, 4, 32]))
                                nc.vector.tensor_tensor(
                                    payex[:, t, 0:pay_w].rearrange(
                                        "p (a b) -> p a b", a=4),
                                    zin, exv, ALU.mult)
                            else:
                                zin = gt[:, col, 0:out_dim]
                                exv = (payex[:, t, pay_w:mw]
                                       .broadcast_to([P, out_dim]))
                                nc.vector.tensor_tensor(
                                    payex[:, t, 0:pay_w], zin, exv, ALU.mult)
                        for t in range(T):
                            nc.tensor.matmul(psz[:], ident[:], payex[:, t, :],
                                             start=(t == 0), stop=(t == T - 1))
                        # epilogue
                        den = wpool.tile([P, nh], F32, tag="den")
                        nc.vector.tensor_scalar_add(den[:], psz[:, pay_w:mw],
                                                    1e-30)
                        rec = wpool.tile([P, nh], F32, tag="rec")
                        nc.vector.reciprocal(rec[:], den[:])
                        if layer == 1:
                            h1 = wpool.tile([P, P], F32, tag="h1")
                            rv = (rec[:].unsqueeze(2)
                                  .broadcast_to([P, 4, 32]))
                            nc.vector.tensor_tensor(
                                h1[:].rearrange("p (a b) -> p a b", a=4),
                                psz[:, 0:P].rearrange("p (a b) -> p a b", a=4),
                                rv, ALU.mult)
                            # elu(x) = max(x, exp(min(x,0)) - 1)
                            mn = wpool.tile([P, P], F32, tag="mn")
                            nc.vector.tensor_scalar_min(mn[:], h1[:], 0.0)
                            em = wpool.tile([P, P], F32, tag="em")
                            nc.scalar.activation(em[:], mn[:], AF.Exp)
                            h1e = wpool.tile([P, P], F32, tag="h1e")
                            nc.vector.scalar_tensor_tensor(
                                h1e[:], em[:], -1.0, h1[:], ALU.add, ALU.max)
                            # z2 = h1e @ [W2 | W2 a2s | W2 a2d]
                            pst2 = pspool.tile([P, P], F32, tag="pst2")
                            nc.tensor.matmul(pst2[:], h1e[:], identf[:],
                                             is_transpose=True)
                            h1T = wpool.tile([P, P], F32, tag="h1T")
                            nc.vector.tensor_copy(h1T[:], pst2[:])
                            psz2 = pspool2.tile([P, out_dim + 2], F32,
                                                tag="psz2")
                            nc.tensor.matmul(psz2[:], h1T[:], stat2[:])
                            # layer-2 row: z2 as bf16 in bytes [0,128),
                            # es2/ed2 f32 at f32 cols 32:34 (256B rows)
                            row2 = wpool.tile([P, 64], F32, tag="row2")
                            r2bf = row2.bitcast(BF16)
                            nc.vector.tensor_copy(r2bf[:, 0:out_dim],
                                                  psz2[:, 0:out_dim])
                            nc.scalar.copy(row2[:, 32:34],
                                           psz2[:, out_dim:out_dim + 2])
                            nc.vector.tensor_copy(scores2[:, k:k + 1],
                                                  psz2[:, out_dim + 1:
                                                       out_dim + 2])
                            nc.sync.dma_start(slice2[k * P:(k + 1) * P, 0:34],
                                              row2[:, 0:34])
                        else:
                            orow = wpool.tile([P, out_dim], F32, tag="orow")
                            rv = rec[:].broadcast_to([P, out_dim])
                            nc.vector.tensor_tensor(orow[:], psz[:, 0:out_dim],
                                                    rv, ALU.mult)
                            nc.sync.dma_start(
                                out_dram[k * P:(k + 1) * P, :], orow[:])

            if phase == "z1":
                nc.sync.dma_start(out_dram[0:npad, :],
                                  slice1[0:npad, 0:out_dim])
            elif phase == "e1":
                edge_phase(1, table1, scores1)
                nc.sync.dma_start(out_dram[0:npad, :],
                                  slice2[0:npad, 0:out_dim])
            else:
                edge_phase(1, table1, scores1)
                nc.sync.dma_start(slice2[npad:npad + 1, :], dum_in[:, 0:64])
                if nocc:
                    pass
                else:
                    nc.gpsimd.collective_compute(
                        "AllGather", ALU.bypass,
                        replica_groups=[list(range(num_devices))],
                        ins=[slice2[0:slice_n, :].opt()],
                        outs=[table2[0:tbl_n, :].opt()])
                edge_phase(2, table2, scores2)

    nc.compile()
    return nc


# --------------------------------------------------------------------------
# host wrapper
# --------------------------------------------------------------------------

def make_inputs(pl, h, W1, a1, W2, a2, in_dim, hid, heads, out_dim):
    n_nodes = h.shape[0]
    npc, npad = pl.npc, pl.npad
    nd = heads * hid
    # stationaries
    st1 = np.zeros((in_dim, 136), np.float32)
    for hh in range(heads):
        st1[:, hh * hid:(hh + 1) * hid] = W1[hh]
        st1[:, 128 + hh] = W1[hh] @ a1[hh, :hid]
        st1[:, 132 + hh] = W1[hh] @ a1[hh, hid:]
    st2 = np.zeros((nd, out_dim + 2), np.float32)
    st2[:, :out_dim] = W2[0]
    st2[:, out_dim] = W2[0] @ a2[0, :out_dim]
    st2[:, out_dim + 1] = W2[0] @ a2[0, out_dim:]
    dummy = np.zeros((1, 128), np.float32)
    dummy[0, 64:72] = -1e30
    dummy[0, 32] = -1e30   # layer-2 es slot (256B rows)
    identb = np.eye(128, dtype=BF)
    identf = np.eye(128, dtype=np.float32)

    ncols_lo = int(pl.tlo.sum()) * 8
    ncols_hi = int(pl.thi.sum()) * 8
    in_maps = []
    st1b = st1.astype(BF)
    for c in range(C):
        hp = np.zeros((npad, in_dim), np.float32)
        valid = pl.perm[c] < npc
        hp[valid] = h[c * npc + pl.perm[c][valid]]
        ht = np.ascontiguousarray(hp.T).astype(BF)
        ilo = wrap_idx(pl.idx_lo[c]) if ncols_lo else \
            np.zeros((128, 8), np.int16)
        ihi = wrap_idx(pl.idx_hi[c]) if ncols_hi else \
            np.zeros((128, 8), np.int16)
        in_maps.append({
            "ht": ht, "idx_lo": ilo, "idx_hi": ihi,
            "stat1": st1b, "stat2": st2, "ident": identb, "identf": identf,
            "dummyrow": dummy,
        })
    return in_maps


def unpermute(pl, outs, n_nodes, out_dim):
    npc = pl.npc
    full = np.zeros((n_nodes, out_dim), np.float32)
    for c in range(C):
        valid = pl.perm[c] < npc
        full[c * npc + pl.perm[c][valid]] = outs[c][valid]
    return full


def gat_run(h, src, dst, W1, a1, W2, a2, runner):
    """Full pipeline; `runner(nc, in_maps)` -> list of per-core {'out': arr}."""
    n_nodes, in_dim = h.shape
    heads, _, hid = W1.shape
    out_dim = W2.shape[2]
    pl = build_plan(np.asarray(src), np.asarray(dst), n_nodes)
    nc = build_program(pl, in_dim, hid, heads, out_dim)
    in_maps = make_inputs(pl, np.asarray(h, np.float32), np.asarray(W1),
                          np.asarray(a1), np.asarray(W2), np.asarray(a2),
                          in_dim, hid, heads, out_dim)
    outs = runner(nc, in_maps)
    return unpermute(pl, [o["out"] for o in outs], n_nodes, out_dim)


def hw_runner(nc, in_maps, trace=None):
    import os
    from concourse.bass_utils import run_bass_kernel_spmd
    if trace is None:
        trace = os.environ.get("GAT_TRACE", "0") == "1"
    res = run_bass_kernel_spmd(nc, in_maps, core_ids=list(range(C)),
                               trace=trace)
    hw_runner.last = res
    return res.results


def kernel(**inputs):
    out = gat_run(inputs["h"], inputs["src"], inputs["dst"], inputs["W1"],
                  inputs["a1"], inputs["W2"], inputs["a2"], hw_runner)
    return out

